# revision 16
# baseline (speedup 1.0000x reference)
"""PointPillars Trainium2 kernel: pillar MLP + masked-max + BEV scatter + 2 convs + head.

v3: dense pillar packing + fp16 + software-pipelined emission.
  - Dense pillar packing into 128-pillar blocks with a *static* greedy block
    assignment (computed from all 8 cores' data at prep time; one SPMD program;
    per-core host-precomputed one-hot planes do the data-dependent part).
  - PFN: two points stacked per rhs column (K=16 block-diag weights, two
    tile-position row streams) -> 2-bank PSUM tile, one tensor_reduce max per
    64 pillars, transpose + tensor_tensor max -> parity-masked embT.
  - Scatter: one matmul per (pair, block): lhsT = [even-masked | odd-masked]
    embT halves, rhs = DMA'd one-hot; even canvas row lands in psum partitions
    0:64, odd in 64:128; one copy per pair into the phase-split canvas.
  - conv1 reads the pair-stacked phase-split canvas: dy0+dy1 taps fuse into
    K=128 matmuls, stride-2 x handled by phase split (contiguous rhs).
  - Emission is interleaved (pfn block -> ready scatter pairs -> ready conv1
    -> ready conv2+head chunks) so the TensorE queue stays dense and the HAM
    clock gate reaches 2.4 GHz early instead of idling at 1.2 GHz.
"""

import os
import sys
from contextlib import ExitStack

sys.path.insert(0, "/opt/trn_rl_repo")

import numpy as np

import concourse.bass as bass
import concourse.tile as tile
from concourse import bacc, mybir
from concourse.masks import make_identity
from concourse.bass_utils import run_bass_kernel_spmd

F16 = mybir.dt.float16
F32 = mybir.dt.float32
NPF16 = np.float16

NCORES = 8
H = W = 512
P = 32            # points per pillar
CE = 64           # embedding channels
ROWS = 70         # canvas rows kept per core (conv1 reads rows 0..68)
PAIRS = 35        # canvas row pairs
C1R = 34          # conv1 output rows (32 owned + 1 halo each side)
OUTR = 32         # owned output rows per core
CW = 258          # canvas phase width: 256 data + 2 zero pad


# ----------------------------------------------------------------------------
# device program (structure S is data-derived but identical for all 8 cores)
# ----------------------------------------------------------------------------

def _build_program(S):
    nblk = S["nblk"]
    combos = S["combos"]          # [(pair, block)] ordered by pair
    ncombo = len(combos)
    npill = nblk * 128
    npt = npill // 64             # psum tiles (64 pillars each)
    scols = npill * 8             # per-stream rhs columns

    by_pair = {}
    for k, (pr, blk) in enumerate(combos):
        by_pair.setdefault(pr, []).append((k, blk))
    # pair pr is ready once its last block's embT exists
    pair_maxblk = {pr: max(b for _, b in cs) for pr, cs in by_pair.items()}

    nc = bacc.Bacc(None, target_bir_lowering=False, debug=False)

    x2a = nc.dram_tensor("x2a", [16, scols], F16, kind="ExternalInput")
    x2b = nc.dram_tensor("x2b", [16, scols], F16, kind="ExternalInput")
    w2 = nc.dram_tensor("w2", [64, 128], F16, kind="ExternalInput")
    ohs = nc.dram_tensor("ohs", [128, ncombo * 512], F16, kind="ExternalInput")
    pmask = nc.dram_tensor("pmask", [128, 2 * nblk], F32, kind="ExternalInput")
    wc1p = nc.dram_tensor("wc1p", [128, 3 * 128], F16, kind="ExternalInput")
    wc1s = nc.dram_tensor("wc1s", [64, 3 * 128], F16, kind="ExternalInput")
    b1v = nc.dram_tensor("b1v", [128, 1], F32, kind="ExternalInput")
    wc2 = nc.dram_tensor("wc2", [128, 9 * 128], F16, kind="ExternalInput")
    b2v = nc.dram_tensor("b2v", [128, 1], F32, kind="ExternalInput")
    whd = nc.dram_tensor("whd", [128, 34], F16, kind="ExternalInput")
    bhd = nc.dram_tensor("bhd", [34, 1], F32, kind="ExternalInput")
    rmask = nc.dram_tensor("rmask", [128, 2], F32, kind="ExternalInput")
    out = nc.dram_tensor("out", [34, OUTR, 256], F32, kind="ExternalOutput")
    debug = os.environ.get("KDEBUG", "0") == "1"
    if debug:
        d_embT = nc.dram_tensor("d_embT", [128, nblk, 2, CE], F16, kind="ExternalOutput")
        d_canvas = nc.dram_tensor("d_canvas", [128, PAIRS, 2, CW], F16, kind="ExternalOutput")
        d_out1 = nc.dram_tensor("d_out1", [128, C1R, 258], F16, kind="ExternalOutput")

    with tile.TileContext(nc) as tc, ExitStack() as ctx:
        const = ctx.enter_context(tc.tile_pool(name="const", bufs=1))
        big = ctx.enter_context(tc.tile_pool(name="big", bufs=1))
        sc = ctx.enter_context(tc.tile_pool(name="scratch", bufs=3))
        ps = ctx.enter_context(tc.tile_pool(name="psum", bufs=2, space="PSUM"))

        # ---- constants in (sync queue; x2 streams + one-hots on gpsimd) ----
        w2_sb = const.tile([64, 128], F16)
        nc.sync.dma_start(w2_sb[:], w2[:])
        pm_sb = const.tile([128, 2 * nblk], F32)
        nc.sync.dma_start(pm_sb[:], pmask[:])
        wc1p_sb = const.tile([128, 3 * 128], F16)
        nc.sync.dma_start(wc1p_sb[:], wc1p[:])
        wc1s_sb = const.tile([64, 3 * 128], F16)
        nc.sync.dma_start(wc1s_sb[:], wc1s[:])
        b1_sb = const.tile([128, 1], F32)
        nc.sync.dma_start(b1_sb[:], b1v[:])
        wc2_sb = const.tile([128, 9 * 128], F16)
        nc.sync.dma_start(wc2_sb[:], wc2[:])
        b2_sb = const.tile([128, 1], F32)
        nc.sync.dma_start(b2_sb[:], b2v[:])
        whd_sb = const.tile([128, 34], F16)
        nc.sync.dma_start(whd_sb[:], whd[:])
        bhd_sb = const.tile([34, 1], F32)
        nc.sync.dma_start(bhd_sb[:], bhd[:])
        rmask_sb = const.tile([128, 2], F32)
        nc.sync.dma_start(rmask_sb[:], rmask[:])

        ident = const.tile([128, 128], F16)
        make_identity(nc, ident[:])
        # ~3.4us of back-to-back dummy matmuls to flip the HAM clock gate to
        # K=8/8 (2.4 GHz) before the real pipeline starts
        prime = ps.tile([128, 128], F32, tag="mm", bufs=5)
        for _ in range(40):
            nc.tensor.matmul(prime[:], lhsT=ident[:], rhs=ident[:],
                             start=True, stop=True)

        # ---- big buffers ----
        x2sb = big.tile([64, scols], F16)        # partitions 0:16 = A, 32:48 = B
        embT = big.tile([128, nblk, 2, CE], F16)  # parity-masked [pillar, ch]
        canvas2 = big.tile([128, PAIRS, 2, CW], F16)
        out1 = big.tile([128, C1R, 258], F16)

        # ---- stream inputs in (8 chunks per stream for pipelining) ----
        ck = scols // 8
        for k in range(8):
            nc.gpsimd.dma_start(x2sb[0:16, k * ck : (k + 1) * ck],
                                x2a[:, k * ck : (k + 1) * ck])
            nc.gpsimd.dma_start(x2sb[32:48, k * ck : (k + 1) * ck],
                                x2b[:, k * ck : (k + 1) * ck])

        nc.vector.memset(canvas2[:, :, :, 256:258], 0.0)
        nc.vector.memset(out1[:, :, 0:1], 0.0)
        nc.vector.memset(out1[:, :, 257:258], 0.0)

        # ---- phase emitters ----------------------------------------------
        def emit_pfn_block(b):
            # two psum tiles (2b, 2b+1) -> hm [128,128] -> embT block b
            hm = sc.tile([128, 128], F16, tag="hm", bufs=2, name=f"hm{b}")
            for half in (0, 1):
                t = 2 * b + half
                for s, p0 in enumerate((0, 32)):
                    pt = ps.tile([128, 32, 16], F32, tag="pfn", bufs=3,
                                 name=f"pt{t}_{s}")
                    nc.tensor.matmul(
                        pt[:], lhsT=w2_sb[p0 : p0 + 16, :],
                        rhs=x2sb[p0 : p0 + 16, 512 * t : 512 * t + 512],
                        start=True, stop=True,
                    )
                    nc.vector.tensor_reduce(
                        hm[:, 64 * half + 32 * s : 64 * half + 32 * s + 32], pt[:],
                        axis=mybir.AxisListType.X, op=mybir.AluOpType.max,
                    )
            ptr = ps.tile([128, 128], F16, tag="mm", bufs=5, name=f"ptr{b}")
            nc.tensor.transpose(ptr[:], hm[:], ident[:])
            trb = sc.tile([128, CE], F16, tag="trb", name=f"trb{b}")
            nc.scalar.copy(trb[:], ptr[:, 64:128])
            emu = sc.tile([128, CE], F16, tag="emu", name=f"emu{b}")
            nc.vector.tensor_tensor(
                emu[:], ptr[:, 0:64], trb[:], op=mybir.AluOpType.max)
            nc.vector.tensor_scalar_mul(
                embT[:, b, 0, :], emu[:], pm_sb[:, 2 * b : 2 * b + 1])
            nc.vector.tensor_scalar_mul(
                embT[:, b, 1, :], emu[:], pm_sb[:, 2 * b + 1 : 2 * b + 2])

        def emit_scat(pr):
            cs = by_pair[pr]
            pp = ps.tile([128, 2, 256], F32, tag="mm", bufs=5, name=f"pp{pr}")
            for j, (k, blk) in enumerate(cs):
                oh = sc.tile([128, 512], F16, tag="oh", bufs=6, name=f"oh{k}")
                eng = nc.gpsimd if (k % 2 == 0) else nc.sync
                eng.dma_start(oh[:], ohs[:, 512 * k : 512 * (k + 1)])
                nc.tensor.matmul(
                    pp[:], lhsT=embT[:, blk, :, :], rhs=oh[:],
                    start=(j == 0), stop=(j == len(cs) - 1),
                )
            nc.scalar.activation(canvas2[:, pr, :, 0:256], pp[:],
                                 mybir.ActivationFunctionType.Relu)

        # conv1: out row o reads canvas rows 2o..2o+2, col j reads 2j..2j+2
        # dx tap -> (phase, j0): x=2j+dx: dx0->ph0[j], dx1->ph1[j], dx2->ph0[j+1]
        DXS = ((0, 0), (1, 0), (0, 1))

        def emit_conv1(chk):
            o = 2 * chk
            pc = ps.tile([128, 2, 256], F32, tag="mm", bufs=5, name=f"pc{chk}")
            for dx, (phx, j0) in enumerate(DXS):
                nc.tensor.matmul(
                    pc[:], lhsT=wc1p_sb[:, 128 * dx : 128 * (dx + 1)],
                    rhs=canvas2[:, o : o + 2, phx, j0 : j0 + 256],
                    start=(dx == 0), stop=False,
                )
            for dx, (phx, j0) in enumerate(DXS):
                nc.tensor.matmul(
                    pc[:], lhsT=wc1s_sb[:, 128 * dx : 128 * (dx + 1)],
                    rhs=canvas2[0:64, o + 1 : o + 3, phx, j0 : j0 + 256],
                    start=False, stop=(dx == 2),
                )
            nc.scalar.activation(
                out1[:, o : o + 2, 1:257], pc[:],
                mybir.ActivationFunctionType.Relu, bias=b1_sb[:],
            )
            # conv2 SAME-padding halo rows are zero at the global edges
            if chk == 0:
                nc.vector.tensor_scalar_mul(
                    out1[:, 0:1, :], out1[:, 0:1, :], rmask_sb[:, 0:1])
            if chk == C1R // 2 - 1:
                nc.vector.tensor_scalar_mul(
                    out1[:, 33:34, :], out1[:, 33:34, :], rmask_sb[:, 1:2])

        def emit_conv2(chk):
            o = 2 * chk
            pc2 = ps.tile([128, 2, 256], F32, tag="mm", bufs=5, name=f"pc2_{chk}")
            for k in range(9):
                dy, dx = k // 3, k % 3
                nc.tensor.matmul(
                    pc2[:], lhsT=wc2_sb[:, 128 * k : 128 * (k + 1)],
                    rhs=out1[:, o + dy : o + dy + 2, dx : dx + 256],
                    start=(k == 0), stop=(k == 8),
                )
            o2c = sc.tile([128, 2, 256], F16, tag="o2c", name=f"o2c{chk}")
            nc.scalar.activation(
                o2c[:], pc2[:], mybir.ActivationFunctionType.Relu, bias=b2_sb[:],
            )
            if debug and chk == 0:
                nc.sync.dma_start(d_embT[:], embT[:])
                nc.sync.dma_start(d_canvas[:], canvas2[:])
                nc.sync.dma_start(d_out1[:], out1[:])
            ph2 = ps.tile([34, 2, 256], F32, tag="mm", bufs=5, name=f"ph2_{chk}")
            nc.tensor.matmul(ph2[:], lhsT=whd_sb[:], rhs=o2c[:],
                             start=True, stop=True)
            hst = sc.tile([34, 2, 256], F32, tag="hst", name=f"hst{chk}")
            nc.scalar.add(hst[:], ph2[:], bhd_sb[:])
            nc.sync.dma_start(out[:, o : o + 2, :], hst[:])

        # ---- interleaved pipeline emission --------------------------------
        # conv1 chunk c needs canvas pairs 2c..2c+2; conv2 chunk c needs out1
        # rows 2c..2c+3 i.e. conv1 chunks 0..c+1 (and the rmask fixups).
        ns = 0   # next scatter pair
        c1 = 0   # next conv1 chunk
        c2 = 0   # next conv2 chunk

        def drain(blocks_done, pairs_done):
            nonlocal ns, c1, c2
            while ns < PAIRS and pair_maxblk[ns] < blocks_done:
                emit_scat(ns)
                ns += 1
            while c1 < C1R // 2 and 2 * c1 + 2 < ns:
                emit_conv1(c1)
                c1 += 1
            while c2 < OUTR // 2 and c2 + 2 <= c1:
                emit_conv2(c2)
                c2 += 1

        for b in range(nblk):
            emit_pfn_block(b)
            drain(b + 1, ns)
        while ns < PAIRS or c1 < C1R // 2 or c2 < OUTR // 2:
            if ns < PAIRS:
                emit_scat(ns)
                ns += 1
            else:
                drain(nblk, ns)
                break
            drain(nblk, ns)

    nc.compile()
    return nc


# ----------------------------------------------------------------------------
# host-side prep
# ----------------------------------------------------------------------------

def _pack_blocks(rowcnt):
    """Static greedy packing of rows into 128-pillar blocks, shared by all
    cores.  Rows may split across a block boundary (every core splits/pads at
    the same block).  Returns (nblk, row_blocks) with row_blocks[r] = [b] or
    [b, b+1]."""
    ncore, nrows = rowcnt.shape
    fill = np.zeros(ncore, np.int64)
    blk = 0
    row_blocks = {}
    for r in range(nrows):
        c = rowcnt[:, r]
        if (fill + c).max() <= 128:
            row_blocks[r] = [blk]
            fill = fill + c
        else:
            row_blocks[r] = [blk, blk + 1]
            fill = np.maximum(fill + c - 128, 0)
            blk += 1
        assert fill.max() <= 128
    return blk + 1, row_blocks


def _prep_inputs(pillar_features, mask, coords, w_pfn, b_pfn,
                 w1, b1, w2, b2, w_cls, b_cls, w_box, b_box):
    pf = np.asarray(pillar_features, np.float32)
    mk = np.asarray(mask, bool)
    xy = np.asarray(coords)
    x, y = xy[:, 0].astype(np.int64), xy[:, 1].astype(np.int64)

    valid = (x >= 0) & (x < W) & (y >= 0) & (y < H)
    lin = y * W + x
    # last-wins dedup among valid pillars (matches XLA scatter .set order)
    vidx = np.nonzero(valid)[0]
    order = vidx[np.argsort(lin[vidx], kind="stable")]
    ls = lin[order]
    is_last = np.ones(len(order), bool)
    if len(order) > 1:
        is_last[:-1] = ls[1:] != ls[:-1]
    keep = order[is_last]
    keep = keep[mk[keep].any(1)]

    mkf = mk[keep].astype(np.float32)                      # (k, 32)
    x8 = np.concatenate([pf[keep] * mkf[:, :, None], mkf[:, :, None]], axis=2)
    kx, ky = x[keep], y[keep]

    # per-core row selection + static block structure
    sel_r, sel_x8, sel_cx = [], [], []
    rowcnt = np.zeros((NCORES, ROWS), np.int64)
    for i in range(NCORES):
        y0 = 64 * i - 2
        sel = (ky >= y0) & (ky < y0 + ROWS - 1)  # rows 0..68 used by conv1
        r = (ky[sel] - y0).astype(np.int64)
        o2 = np.argsort(r, kind="stable")
        sel_r.append(r[o2])
        sel_cx.append(kx[sel][o2])
        sel_x8.append(x8[sel][o2])
        rowcnt[i] = np.bincount(r[o2], minlength=ROWS)

    nblk, row_blocks = _pack_blocks(rowcnt)
    npill = nblk * 128
    scols = npill * 8

    # pair-level combos: union of the two rows' blocks (contiguous)
    combos = []
    for pr in range(PAIRS):
        bs = sorted(set(row_blocks[2 * pr] + row_blocks[2 * pr + 1]))
        for b in range(bs[0], bs[-1] + 1):
            combos.append((pr, b))
    combo_idx = {c: k for k, c in enumerate(combos)}
    ncombo = len(combos)
    S = {"nblk": nblk, "combos": tuple(combos)}

    in_maps = []
    pmask_shared = None
    for i in range(NCORES):
        r_s, cx_s, xf_s = sel_r[i], sel_cx[i], sel_x8[i]
        m = len(r_s)
        # dense index per pillar under the static block layout
        d = np.zeros(m, np.int64)
        pos = 0
        row_start = np.searchsorted(r_s, np.arange(ROWS + 1))
        for r in range(ROWS):
            idx = row_start[r]
            cnt = row_start[r + 1] - idx
            bs = row_blocks[r]
            if len(bs) == 1:
                assert bs[0] * 128 <= pos and pos + cnt <= (bs[0] + 1) * 128
                d[idx : idx + cnt] = pos + np.arange(cnt)
                pos += cnt
            else:
                bound = (bs[0] + 1) * 128
                take = min(cnt, bound - pos)
                d[idx : idx + take] = pos + np.arange(take)
                d[idx + take : idx + cnt] = bound + np.arange(cnt - take)
                pos = bound + (cnt - take)
        assert pos <= npill, f"core {i}: {pos} > {npill}"

        # x2 stream layout: fill F = d//32, slot p = d%32, pair j: points j, j+16
        F, p = d // 32, d % 32
        scol = 512 * (F // 2) + p * 16
        xa = np.zeros((16, scols), np.float32)
        xb = np.zeros((16, scols), np.float32)
        cols = scol[:, None] + np.arange(16)[None, :]
        sa = (F % 2 == 0)
        featA = xf_s[:, 0:16, :].transpose(2, 0, 1)         # (8, m, 16)
        featB = xf_s[:, 16:32, :].transpose(2, 0, 1)
        xa[0:8, cols[sa]] = featA[:, sa]
        xa[8:16, cols[sa]] = featB[:, sa]
        xb[0:8, cols[~sa]] = featA[:, ~sa]
        xb[8:16, cols[~sa]] = featB[:, ~sa]

        # one-hot planes per (pair, block) combo + parity masks per block
        tgt = (cx_s % 2) * 256 + cx_s // 2
        blk_of = d // 128
        part_of = d % 128
        kidx = np.array([combo_idx[(int(r_s[j]) // 2, int(blk_of[j]))]
                         for j in range(m)], np.int64)
        ohp = np.zeros((128, ncombo, 512), NPF16)
        ohp[part_of, kidx, tgt] = 1.0
        pm = np.zeros((128, 2 * nblk), np.float32)
        pm[part_of, 2 * blk_of + (r_s % 2)] = 1.0

        rm = np.ones((128, 2), np.float32)
        if i == 0:
            rm[:, 0] = 0.0
        if i == NCORES - 1:
            rm[:, 1] = 0.0

        in_maps.append({
            "x2a": xa.astype(NPF16),
            "x2b": xb.astype(NPF16),
            "ohs": ohp.reshape(128, ncombo * 512),
            "pmask": pm,
            "rmask": rm,
        })

    # shared weights
    w8 = np.concatenate([np.asarray(w_pfn, np.float32),
                         np.asarray(b_pfn, np.float32)[None, :]], 0)  # (8, 64)
    w2f = np.zeros((64, 128), np.float32)
    w2f[0:8, 0:64] = w8
    w2f[8:16, 64:128] = w8
    w2f[32:40, 0:64] = w8
    w2f[40:48, 64:128] = w8

    w1a = np.asarray(w1, np.float32)                        # (128, 64, 3, 3)
    wc1p = np.zeros((128, 3 * 128), np.float32)
    wc1s = np.zeros((64, 3 * 128), np.float32)
    for dx in range(3):
        wc1p[0:64, 128 * dx : 128 * (dx + 1)] = w1a[:, :, 0, dx].T
        wc1p[64:128, 128 * dx : 128 * (dx + 1)] = w1a[:, :, 1, dx].T
        wc1s[:, 128 * dx : 128 * (dx + 1)] = w1a[:, :, 2, dx].T

    wc2 = np.ascontiguousarray(
        np.asarray(w2, np.float32).transpose(2, 3, 1, 0).reshape(9, 128, 128)
        .transpose(1, 0, 2).reshape(128, 9 * 128)
    )
    whd = np.ascontiguousarray(np.concatenate(
        [np.asarray(w_cls, np.float32)[:, :, 0, 0],
         np.asarray(w_box, np.float32)[:, :, 0, 0]], 0).T)
    bhd = np.concatenate([np.asarray(b_cls, np.float32),
                          np.asarray(b_box, np.float32)])[:, None].astype(np.float32)
    b1c = np.asarray(b1, np.float32)[:, None]
    b2c = np.asarray(b2, np.float32)[:, None]

    shared = {
        "w2": w2f.astype(NPF16), "wc1p": wc1p.astype(NPF16),
        "wc1s": wc1s.astype(NPF16), "b1v": b1c, "wc2": wc2.astype(NPF16),
        "b2v": b2c, "whd": whd.astype(NPF16), "bhd": bhd,
    }
    for mmap in in_maps:
        mmap.update(shared)
    return in_maps, S


_CACHE = {}


def kernel(pillar_features, mask, coords, H=None, W=None,
           w_pfn=None, b_pfn=None, w1=None, b1=None, w2=None, b2=None,
           w_cls=None, b_cls=None, w_box=None, b_box=None):
    in_maps, S = _prep_inputs(pillar_features, mask, coords, w_pfn, b_pfn,
                              w1, b1, w2, b2, w_cls, b_cls, w_box, b_box)
    key = (S["nblk"], S["combos"])
    if _CACHE.get("key") != key:
        _CACHE["nc"] = _build_program(S)
        _CACHE["key"] = key
    nc = _CACHE["nc"]

    trace = os.environ.get("KERNEL_TRACE", "0") == "1"
    res = run_bass_kernel_spmd(nc, in_maps, core_ids=list(range(NCORES)),
                               trace=trace)
    if trace and res.exec_time_ns is not None:
        print(f"HW exec time: {res.exec_time_ns} ns")
        _CACHE["exec_time_ns"] = res.exec_time_ns

    full = np.zeros((34, 256, 256), np.float32)
    for i in range(NCORES):
        full[:, 32 * i : 32 * i + 32, :] = res.results[i]["out"]
    return full[None]
